# revision 44
# baseline (speedup 1.0000x reference)
"""Distributed Trainium2 Bass kernel for the DriftingField problem.

Math (reference):
    targets = [gen; pos]                         # [T, D], T = G + P
    d2[i,j] = |gen_i|^2 + |tgt_j|^2 - 2 gen_i.tgt_j
    dist    = sqrt(d2) / sqrt(D); dist[i,i] = 1e6 (gen block diag)
    K       = exp(-dist / TEMP)                  # [G, T]
    nk      = K / sqrt(max(rs_i * cs_j, 1e-12))
    out     = (nk[:,G:] * s_gen) @ pos - (nk[:,:G] * s_pos) @ gen
              s_gen_i = sum_{j<G} nk[i,j], s_pos_i = sum_{j>=G} nk[i,j]

Key numerical fact: for this problem's data (randn features, D=1024,
TEMP=0.05) every K entry is ~exp(-28) ~ 5e-13, so rs_i*cs_j ~ 1e-17 <<
1e-12 and the max() clamp is ALWAYS active: the normalizer is the
constant 1e-6. Then
    out = 1e12 * [ rg_i * (K[:,G:] @ pos) - rp_i * (K[:,:G] @ gen) ]
with rg/rp the per-row sums of K over the gen/pos halves. No cross-core
reduction is needed: the G-sharding is embarrassingly parallel. The
kernel still emits per-core row/column sums; the host checks
max(rs)*max(cs) stays safely under the clamp and falls back to an exact
numpy evaluation if any input ever leaves the clamped regime.

Layout per core (512 gen rows): K^T tiles [j partitions, i free] so that
  - b2_j rides the ACT sqrt bias (per-partition),
  - column-sum partials come free via the Exp activation's accum_out,
  - the output matmuls need no transposition of K.
a2_i is folded into the distance matmul as a K=2 augmentation (hi/lo
bf16 split of a2 against a ones stationary); the gen-block diagonal
DIAG_FILL is folded in as a K=128 matmul of the identity against a
per-core sliding-window mask whose window offset (T - js*128) makes the
diagonal line land js-independently (q = T + p - c*RPC).
Host-side prep (layout/dtype only, no FLOPs moved off-device): concat
targets, transpose, bf16 casts of the matmul operands, the fill mask.
"""

import numpy as np
from contextlib import ExitStack

import concourse.bass as bass
import concourse.bacc as bacc
import concourse.mybir as mybir
import concourse.tile as tile
from concourse.bass_utils import run_bass_kernel_spmd
from concourse.masks import make_identity

F32 = mybir.dt.float32
BF16 = mybir.dt.bfloat16
AF = mybir.ActivationFunctionType

NCORES = 8
TEMP = 0.05
BIG = 1.0e12
CLAMP = 1.0e-12          # reference: max(rs*cs, 1e-12)
INV_NORM2 = 1.0 / CLAMP  # 1e12, the (1/normalizer)^2 when clamped

TRACE = False
LAST_RESULT = None


def build_nc(G, P, D):
    T = G + P
    RPC = G // NCORES          # gen rows per core
    NJ = T // 128              # j-subtiles (target rows)
    NI = RPC // 128            # i-chunks (this core's gen rows)
    ND = D // 128              # d-chunks (feature dim)
    WFILL = T + RPC            # sliding-window fill width
    EXP_SCALE = -1.0 / (TEMP * float(D) ** 0.5)
    NJG = G // 128             # j-subtiles in the gen block

    nc = bacc.Bacc(trn_type="TRN2", num_devices=NCORES)

    gen_rows = nc.dram_tensor("gen_rows", [RPC, D], F32, kind="ExternalInput")
    targets_bf = nc.dram_tensor("targets_bf", [T, D], BF16, kind="ExternalInput")
    targets_T_bf = nc.dram_tensor("targets_T_bf", [D, T], BF16,
                                  kind="ExternalInput")
    fill_wide = nc.dram_tensor("fill_wide", [128, WFILL], BF16,
                               kind="ExternalInput")
    out = nc.dram_tensor("out", [RPC, D], F32, kind="ExternalOutput")
    cs_part = nc.dram_tensor("cs_part", [128, NJ], F32, kind="ExternalOutput")
    rs_out = nc.dram_tensor("rs_out", [128, NI], F32, kind="ExternalOutput")
    a2_dram = nc.dram_tensor("a2_dram", [1, RPC], F32)

    with tile.TileContext(nc) as tc, ExitStack() as ctx:
        const = ctx.enter_context(tc.tile_pool(name="const", bufs=1))
        work = ctx.enter_context(tc.tile_pool(name="work", bufs=3))

        identity = const.tile([128, 128], BF16, tag="identity")
        make_identity(nc, identity)
        ones_bf = const.tile([128, 1], BF16, tag="ones_bf")
        nc.vector.memset(ones_bf, 1.0)

        fill_sb = const.tile([128, WFILL], BF16, tag="fill_sb")
        nc.sync.dma_start(out=fill_sb, in_=fill_wide[:, :])

        b2_sb = const.tile([128, NJ], F32, tag="b2_sb")
        cs_sb = const.tile([128, NJ], F32, tag="cs_sb")
        KT = const.tile([128, NJ, RPC], BF16, tag="KT")
        genT = const.tile([128, ND, RPC], BF16, tag="genT")
        a2_col = const.tile([128, NI], F32, tag="a2_col")
        a2_row = const.tile([1, RPC], F32, tag="a2_row")
        a2_bcast = const.tile([128, RPC], F32, tag="a2_bcast")

        # ---- gen-side prep: a2, cast*(-2), transpose to [d, i] ----
        with tc.tile_pool(name="tpsum", bufs=2, space="PSUM") as tpsum:
            for ic in range(NI):
                gci = work.tile([128, D], F32, tag="f32big")
                nc.sync.dma_start(out=gci, in_=gen_rows[ic * 128:(ic + 1) * 128, :])
                sq_scr = work.tile([128, D], BF16, tag="sqscr")
                nc.scalar.activation(sq_scr, gci, AF.Square,
                                     accum_out=a2_col[:, ic:ic + 1])
                gbf = work.tile([128, D], BF16, tag="bf16big")
                nc.scalar.activation(gbf, gci, AF.Copy, scale=-2.0)
                for dc in range(ND):
                    pt = tpsum.tile([128, 128], BF16, tag="pt")
                    nc.tensor.transpose(pt, gbf[:, dc * 128:(dc + 1) * 128], identity)
                    nc.scalar.copy(genT[:, dc, ic * 128:(ic + 1) * 128], pt)
            # a2 [128, NI] -> row [1, RPC] -> broadcast [128, RPC] via a
            # stride-0 DMA from a DRAM scratch row
            for ic in range(NI):
                nc.sync.dma_start(out=a2_row[0:1, ic * 128:(ic + 1) * 128],
                                  in_=a2_col[:, ic:ic + 1])
            nc.sync.dma_start(out=a2_dram[:, :], in_=a2_row)
            a2d = a2_dram[:, :]
            a2_bc_src = bass.AP(tensor=a2d.tensor, offset=a2d.offset,
                                ap=[[0, 128], a2d.ap[1]])
            nc.sync.dma_start(out=a2_bcast, in_=a2_bc_src)

        # ---- main loop over target row blocks: K^T tiles ----
        # Per group of NG j-subtiles: ACT does only the batched sqrt/exp,
        # DVE does only the b2 square-reduce, PE does everything else
        # (distance matmul + a2 augmentation + diagonal fill matmul).
        NG = 8
        NH = 4  # target-row batch (rows-of-128) for b2 / V loads
        tTb_pool = ctx.enter_context(tc.tile_pool(name="tTb_pool", bufs=2))
        tbf_pool = ctx.enter_context(tc.tile_pool(name="tbf_pool", bufs=2))
        with tc.tile_pool(name="mpsum", bufs=NG, space="PSUM") as mpsum:
            for g in range(NJ // NG):
                j0 = g * NG * 128
                # all 8 d-chunk slices of targets_T for this group: ONE DMA
                tTb = tTb_pool.tile([128, ND, NG * 128], BF16, tag="tTb",
                                    name=f"tTb{g}")
                nc.sync.dma_start(
                    out=tTb,
                    in_=targets_T_bf[:, j0:j0 + NG * 128].rearrange(
                        "(c p) j -> p c j", p=128))
                # b2 via ACT square-reduce, NH rows-of-128 per DMA
                for h in range(NG // NH):
                    jh = j0 + h * NH * 128
                    tbf = tbf_pool.tile([128, NH, D], BF16, tag="tbf",
                                        name=f"tbf{g}_{h}")
                    nc.sync.dma_start(
                        out=tbf,
                        in_=targets_bf[jh:jh + NH * 128, :].rearrange(
                            "(k p) d -> p k d", p=128))
                    for kk in range(NH):
                        js = g * NG + h * NH + kk
                        bscr = work.tile([128, D], BF16, tag="sqscr")
                        nc.scalar.activation(bscr, tbf[:, kk, :], AF.Square,
                                             accum_out=b2_sb[:, js:js + 1])
                pss = []
                for k in range(NG):
                    ps = mpsum.tile([128, RPC], F32, tag="ps", name=f"ps{k}")
                    for dc in range(ND):
                        nc.tensor.matmul(ps,
                                         lhsT=tTb[:, dc, k * 128:(k + 1) * 128],
                                         rhs=genT[:, dc, :],
                                         start=(dc == 0), stop=(dc == ND - 1))
                    pss.append(ps)
                # d2 = (-2 g.t) + a2_i (+BIG on the gen-block diagonal)
                for k in range(NG):
                    js = g * NG + k
                    nc.vector.tensor_add(pss[k], pss[k], a2_bcast)
                    off = T - js * 128
                    nc.vector.tensor_add(pss[k], pss[k], fill_sb[:, off:off + RPC])
                for k in range(NG):
                    js = g * NG + k
                    nc.scalar.activation(pss[k], pss[k], AF.Sqrt,
                                         bias=b2_sb[:, js:js + 1])
                for k in range(NG):
                    js = g * NG + k
                    nc.scalar.activation(KT[:, js, :], pss[k], AF.Exp,
                                         scale=EXP_SCALE,
                                         accum_out=cs_sb[:, js:js + 1])

        rs_sb = const.tile([128, NI], F32, tag="rs_sb")
        alpha = const.tile([128, NI], F32, tag="alpha")
        beta = const.tile([128, NI], F32, tag="beta")
        sgen = const.tile([128, NI, D], BF16, tag="sgen")

        # ---- output matmuls, row sums fused on the same stationaries ----
        # Two passes over V, each covering an ic-pair; within a pass the
        # gen half (S_gen + rg) runs first, is evicted, then the pos half.
        NDH = (D + 511) // 512
        vt_pool = ctx.enter_context(tc.tile_pool(name="vt_pool", bufs=3))
        m2pool = ctx.enter_context(tc.tile_pool(name="m2pool", bufs=3,
                                                space="PSUM"))
        rspool = ctx.enter_context(tc.tile_pool(name="rspool", bufs=2,
                                                space="PSUM"))
        rg_row = const.tile([1, RPC], F32, tag="rg_row")
        rp_row = const.tile([1, RPC], F32, tag="rp_row")
        for half in range(NI // 2):
            ics = [2 * half, 2 * half + 1]
            mg = {ic: m2pool.tile([128, D], F32, tag="m2h", name=f"mg{ic}")
                  for ic in ics}
            if half == 0:
                rgp = rspool.tile([1, RPC], F32, tag="racc", name="rgp")
            for jb in range(NJG // 4):
                vt = vt_pool.tile([128, 4, D], BF16, tag="vt")
                nc.sync.dma_start(
                    out=vt,
                    in_=targets_bf[jb * 512:(jb + 1) * 512, :].rearrange(
                        "(k p) d -> p k d", p=128))
                for kk in range(4):
                    js = jb * 4 + kk
                    for ic in ics:
                        lhs = KT[:, js, ic * 128:(ic + 1) * 128]
                        for dh in range(NDH):
                            d0 = dh * 512
                            d1 = min(D, d0 + 512)
                            nc.tensor.matmul(mg[ic][:, d0:d1], lhsT=lhs,
                                             rhs=vt[:, kk, d0:d1],
                                             start=(js == 0),
                                             stop=(js == NJG - 1))
                    if half == 0:
                        nc.tensor.matmul(rgp, lhsT=ones_bf, rhs=KT[:, js, :],
                                         start=(js == 0), stop=(js == NJG - 1))
            if half == 0:
                # evict 1e12*rg and scatter to per-partition layout
                nc.scalar.activation(rg_row, rgp, AF.Copy, scale=INV_NORM2)
                for ic in range(NI):
                    nc.sync.dma_start(
                        out=beta[:, ic:ic + 1],
                        in_=rg_row[0:1, ic * 128:(ic + 1) * 128])
            for ic in ics:
                nc.scalar.copy(sgen[:, ic, :], mg[ic])
            mp = {ic: m2pool.tile([128, D], F32, tag="m2h", name=f"mp{ic}")
                  for ic in ics}
            if half == 0:
                rpp = rspool.tile([1, RPC], F32, tag="racc", name="rpp")
            for jb in range(NJG // 4, NJ // 4):
                vt = vt_pool.tile([128, 4, D], BF16, tag="vt")
                nc.sync.dma_start(
                    out=vt,
                    in_=targets_bf[jb * 512:(jb + 1) * 512, :].rearrange(
                        "(k p) d -> p k d", p=128))
                for kk in range(4):
                    js = jb * 4 + kk
                    for ic in ics:
                        lhs = KT[:, js, ic * 128:(ic + 1) * 128]
                        for dh in range(NDH):
                            d0 = dh * 512
                            d1 = min(D, d0 + 512)
                            nc.tensor.matmul(mp[ic][:, d0:d1], lhsT=lhs,
                                             rhs=vt[:, kk, d0:d1],
                                             start=(js == NJG),
                                             stop=(js == NJ - 1))
                    if half == 0:
                        nc.tensor.matmul(rpp, lhsT=ones_bf, rhs=KT[:, js, :],
                                         start=(js == NJG), stop=(js == NJ - 1))
            if half == 0:
                nc.scalar.activation(rp_row, rpp, AF.Copy, scale=INV_NORM2)
                for ic in range(NI):
                    nc.sync.dma_start(
                        out=alpha[:, ic:ic + 1],
                        in_=rp_row[0:1, ic * 128:(ic + 1) * 128])
            # out = beta*S_pos - alpha*S_gen
            # (alpha here = 1e12*rp scales the GEN half; beta = 1e12*rg
            #  scales the POS half, matching the reference's cross pairing)
            for ic in ics:
                t1 = work.tile([128, D], F32, tag="f32big")
                nc.vector.tensor_scalar_mul(t1, mp[ic], beta[:, ic:ic + 1])
                t2 = work.tile([128, D], F32, tag="f32big")
                nc.vector.tensor_scalar_mul(t2, sgen[:, ic, :],
                                            alpha[:, ic:ic + 1])
                ot = work.tile([128, D], F32, tag="f32big")
                nc.vector.tensor_sub(ot, t1, t2)
                nc.sync.dma_start(out=out[ic * 128:(ic + 1) * 128, :], in_=ot)

        # rs = (alpha + beta) / 1e12, both already in SBUF
        nc.vector.tensor_add(rs_sb, alpha, beta)
        nc.vector.tensor_scalar_mul(rs_sb, rs_sb, 1.0 / INV_NORM2)
        nc.sync.dma_start(out=rs_out[:, :], in_=rs_sb)
        nc.sync.dma_start(out=cs_part[:, :], in_=cs_sb)

    nc.compile()
    return nc


def make_in_maps(gen, pos, G, P, D):
    import ml_dtypes
    T = G + P
    RPC = G // NCORES
    WFILL = T + RPC
    targets = np.concatenate([gen, pos], axis=0).astype(np.float32)
    targets_bf = np.ascontiguousarray(targets.astype(ml_dtypes.bfloat16))
    targets_t_bf = np.ascontiguousarray(targets_bf.T)
    in_maps = []
    p = np.arange(128)
    for c in range(NCORES):
        fill = np.zeros((128, WFILL), ml_dtypes.bfloat16)
        # diagonal entries: q = T + p - c*RPC  (js-independent; the per-js
        # window [T - js*128, +RPC) hits it exactly at the gen-block diagonal)
        q = T + p - c * RPC
        fill[p, q] = BIG
        in_maps.append({
            "gen_rows": np.ascontiguousarray(gen[c * RPC:(c + 1) * RPC]).astype(np.float32),
            "targets_bf": targets_bf,
            "targets_T_bf": targets_t_bf,
            "fill_wide": fill,
        })
    return in_maps


def _exact_numpy_reference(gen, pos):
    """Bit-faithful (float64) fallback for inputs outside the clamped regime."""
    G, D = gen.shape
    gen64 = gen.astype(np.float64)
    pos64 = pos.astype(np.float64)
    tgt = np.concatenate([gen64, pos64], 0)
    d2 = (gen64 * gen64).sum(-1)[:, None] + (tgt * tgt).sum(-1)[None, :] \
        - 2.0 * gen64 @ tgt.T
    dist = np.sqrt(np.maximum(d2, 0.0))
    if D > 10:
        dist = dist / np.sqrt(D)
    idx = np.arange(G)
    dist[idx, idx] = 1e6
    k = np.exp(-dist / TEMP)
    rs = k.sum(-1, keepdims=True)
    cs = k.sum(-2, keepdims=True)
    nk = k / np.sqrt(np.maximum(rs * cs, CLAMP))
    pos_c = nk[:, G:] * nk[:, :G].sum(-1, keepdims=True)
    neg_c = nk[:, :G] * nk[:, G:].sum(-1, keepdims=True)
    return (pos_c @ pos64 - neg_c @ gen64).astype(np.float32)


_NC_CACHE = {}


def _get_nc(G, P, D):
    key = (G, P, D)
    if key not in _NC_CACHE:
        _NC_CACHE[key] = build_nc(G, P, D)
    return _NC_CACHE[key]


def kernel(gen_features, pos_features):
    global LAST_RESULT
    gen = np.asarray(gen_features, dtype=np.float32)
    pos = np.asarray(pos_features, dtype=np.float32)
    G, D = gen.shape
    P = pos.shape[0]
    nc = _get_nc(G, P, D)
    in_maps = make_in_maps(gen, pos, G, P, D)
    res = run_bass_kernel_spmd(nc, in_maps, core_ids=list(range(NCORES)),
                               trace=TRACE)
    LAST_RESULT = res
    out = np.concatenate([res.results[c]["out"] for c in range(NCORES)], axis=0)

    # Clamp-regime guard: the device kernel assumes rs_i*cs_j <= 1e-12
    # everywhere (always true for this problem's data). Verify from the
    # device's own row/column sums; fall back to exact evaluation if not.
    cs_glob = sum(res.results[c]["cs_part"] for c in range(NCORES))
    rs_max = max(float(res.results[c]["rs_out"].max()) for c in range(NCORES))
    if rs_max * float(cs_glob.max()) > 0.25 * CLAMP:
        return _exact_numpy_reference(gen, pos)
    return out.astype(np.float32)


# revision 45
# speedup vs baseline: 1.0731x; 1.0731x over previous
"""Distributed Trainium2 Bass kernel for the DriftingField problem.

Math (reference):
    targets = [gen; pos]                         # [T, D], T = G + P
    d2[i,j] = |gen_i|^2 + |tgt_j|^2 - 2 gen_i.tgt_j
    dist    = sqrt(d2) / sqrt(D); dist[i,i] = 1e6 (gen block diag)
    K       = exp(-dist / TEMP)                  # [G, T]
    nk      = K / sqrt(max(rs_i * cs_j, 1e-12))
    out     = (nk[:,G:] * s_gen) @ pos - (nk[:,:G] * s_pos) @ gen
              s_gen_i = sum_{j<G} nk[i,j], s_pos_i = sum_{j>=G} nk[i,j]

Key numerical fact: for this problem's data (randn features, D=1024,
TEMP=0.05) every K entry is ~exp(-28) ~ 5e-13, so rs_i*cs_j ~ 1e-17 <<
1e-12 and the max() clamp is ALWAYS active: the normalizer is the
constant 1e-6. Then
    out = 1e12 * [ rg_i * (K[:,G:] @ pos) - rp_i * (K[:,:G] @ gen) ]
with rg/rp the per-row sums of K over the gen/pos halves. No cross-core
reduction is needed: the G-sharding is embarrassingly parallel. The
kernel still emits per-core row/column sums; the host checks
max(rs)*max(cs) stays safely under the clamp and falls back to an exact
numpy evaluation if any input ever leaves the clamped regime.

Layout per core (512 gen rows): K^T tiles [j partitions, i free] so that
  - b2_j rides the ACT sqrt bias (per-partition),
  - column-sum partials come free via the Exp activation's accum_out,
  - the output matmuls need no transposition of K.
a2_i is folded into the distance matmul as a K=2 augmentation (hi/lo
bf16 split of a2 against a ones stationary); the gen-block diagonal
DIAG_FILL is folded in as a K=128 matmul of the identity against a
per-core sliding-window mask whose window offset (T - js*128) makes the
diagonal line land js-independently (q = T + p - c*RPC).
Host-side prep (layout/dtype only, no FLOPs moved off-device): concat
targets, transpose, bf16 casts of the matmul operands, the fill mask.
"""

import numpy as np
from contextlib import ExitStack

import concourse.bass as bass
import concourse.bacc as bacc
import concourse.mybir as mybir
import concourse.tile as tile
from concourse.bass_utils import run_bass_kernel_spmd
from concourse.masks import make_identity

F32 = mybir.dt.float32
BF16 = mybir.dt.bfloat16
AF = mybir.ActivationFunctionType

NCORES = 8
TEMP = 0.05
BIG = 1.0e12
CLAMP = 1.0e-12          # reference: max(rs*cs, 1e-12)
INV_NORM2 = 1.0 / CLAMP  # 1e12, the (1/normalizer)^2 when clamped

TRACE = False
LAST_RESULT = None


def build_nc(G, P, D):
    T = G + P
    RPC = G // NCORES          # gen rows per core
    NJ = T // 128              # j-subtiles (target rows)
    NI = RPC // 128            # i-chunks (this core's gen rows)
    ND = D // 128              # d-chunks (feature dim)
    WFILL = T + RPC            # sliding-window fill width
    EXP_SCALE = -1.0 / (TEMP * float(D) ** 0.5)
    NJG = G // 128             # j-subtiles in the gen block

    nc = bacc.Bacc(trn_type="TRN2", num_devices=NCORES)

    gen_rows = nc.dram_tensor("gen_rows", [RPC, D], F32, kind="ExternalInput")
    targets_bf = nc.dram_tensor("targets_bf", [T, D], BF16, kind="ExternalInput")
    targets_T_bf = nc.dram_tensor("targets_T_bf", [D, T], BF16,
                                  kind="ExternalInput")
    fill_wide = nc.dram_tensor("fill_wide", [128, WFILL], BF16,
                               kind="ExternalInput")
    out = nc.dram_tensor("out", [RPC, D], F32, kind="ExternalOutput")
    cs_part = nc.dram_tensor("cs_part", [128, NJ], F32, kind="ExternalOutput")
    rs_out = nc.dram_tensor("rs_out", [128, NI], F32, kind="ExternalOutput")
    a2_dram = nc.dram_tensor("a2_dram", [1, RPC], F32)

    with tile.TileContext(nc) as tc, ExitStack() as ctx:
        const = ctx.enter_context(tc.tile_pool(name="const", bufs=1))
        work = ctx.enter_context(tc.tile_pool(name="work", bufs=3))

        identity = const.tile([128, 128], BF16, tag="identity")
        make_identity(nc, identity)
        ones_bf = const.tile([128, 1], BF16, tag="ones_bf")
        nc.vector.memset(ones_bf, 1.0)

        fill_sb = const.tile([128, WFILL], BF16, tag="fill_sb")
        nc.sync.dma_start(out=fill_sb, in_=fill_wide[:, :])

        b2_sb = const.tile([128, NJ], F32, tag="b2_sb")
        cs_sb = const.tile([128, NJ], F32, tag="cs_sb")
        KT = const.tile([128, NJ, RPC], BF16, tag="KT")
        genT = const.tile([128, ND, RPC], BF16, tag="genT")
        a2_col = const.tile([128, NI], F32, tag="a2_col")
        a2_row = const.tile([1, RPC], F32, tag="a2_row")
        a2_bcast = const.tile([128, RPC], F32, tag="a2_bcast")

        # ---- gen-side prep: a2, cast*(-2), transpose to [d, i] ----
        with tc.tile_pool(name="tpsum", bufs=2, space="PSUM") as tpsum:
            for ic in range(NI):
                gci = work.tile([128, D], F32, tag="f32big")
                nc.sync.dma_start(out=gci, in_=gen_rows[ic * 128:(ic + 1) * 128, :])
                sq_scr = work.tile([128, D], BF16, tag="sqscr")
                nc.scalar.activation(sq_scr, gci, AF.Square,
                                     accum_out=a2_col[:, ic:ic + 1])
                gbf = work.tile([128, D], BF16, tag="bf16big")
                nc.scalar.activation(gbf, gci, AF.Copy, scale=-2.0)
                for dc in range(ND):
                    pt = tpsum.tile([128, 128], BF16, tag="pt")
                    nc.tensor.transpose(pt, gbf[:, dc * 128:(dc + 1) * 128], identity)
                    nc.scalar.copy(genT[:, dc, ic * 128:(ic + 1) * 128], pt)
            # a2 [128, NI] -> row [1, RPC] -> broadcast [128, RPC] via a
            # stride-0 DMA from a DRAM scratch row
            for ic in range(NI):
                nc.sync.dma_start(out=a2_row[0:1, ic * 128:(ic + 1) * 128],
                                  in_=a2_col[:, ic:ic + 1])
            nc.sync.dma_start(out=a2_dram[:, :], in_=a2_row)
            a2d = a2_dram[:, :]
            a2_bc_src = bass.AP(tensor=a2d.tensor, offset=a2d.offset,
                                ap=[[0, 128], a2d.ap[1]])
            nc.sync.dma_start(out=a2_bcast, in_=a2_bc_src)

        # ---- main loop over target row blocks: K^T tiles ----
        # Per group of NG j-subtiles: ACT does only the batched sqrt/exp,
        # DVE does only the b2 square-reduce, PE does everything else
        # (distance matmul + a2 augmentation + diagonal fill matmul).
        NG = 8
        NH = 4  # target-row batch (rows-of-128) for b2 / V loads
        tTb_pool = ctx.enter_context(tc.tile_pool(name="tTb_pool", bufs=2))
        tbf_pool = ctx.enter_context(tc.tile_pool(name="tbf_pool", bufs=2))
        with tc.tile_pool(name="mpsum", bufs=NG, space="PSUM") as mpsum:
            for g in range(NJ // NG):
                j0 = g * NG * 128
                # all 8 d-chunk slices of targets_T for this group: ONE DMA
                tTb = tTb_pool.tile([128, ND, NG * 128], BF16, tag="tTb",
                                    name=f"tTb{g}")
                nc.sync.dma_start(
                    out=tTb,
                    in_=targets_T_bf[:, j0:j0 + NG * 128].rearrange(
                        "(c p) j -> p c j", p=128))
                # b2 via ACT square-reduce, NH rows-of-128 per DMA
                for h in range(NG // NH):
                    jh = j0 + h * NH * 128
                    tbf = tbf_pool.tile([128, NH, D], BF16, tag="tbf",
                                        name=f"tbf{g}_{h}")
                    nc.sync.dma_start(
                        out=tbf,
                        in_=targets_bf[jh:jh + NH * 128, :].rearrange(
                            "(k p) d -> p k d", p=128))
                    for kk in range(NH):
                        js = g * NG + h * NH + kk
                        bscr = work.tile([128, D], BF16, tag="sqscr")
                        nc.scalar.activation(bscr, tbf[:, kk, :], AF.Square,
                                             accum_out=b2_sb[:, js:js + 1])
                pss = []
                for k in range(NG):
                    ps = mpsum.tile([128, RPC], F32, tag="ps", name=f"ps{k}")
                    for dc in range(ND):
                        nc.tensor.matmul(ps,
                                         lhsT=tTb[:, dc, k * 128:(k + 1) * 128],
                                         rhs=genT[:, dc, :],
                                         start=(dc == 0), stop=(dc == ND - 1))
                    pss.append(ps)
                # d2 = (-2 g.t) + a2_i (+BIG on the gen-block diagonal)
                for k in range(NG):
                    js = g * NG + k
                    nc.vector.tensor_add(pss[k], pss[k], a2_bcast)
                    off = T - js * 128
                    nc.vector.tensor_add(pss[k], pss[k], fill_sb[:, off:off + RPC])
                for k in range(NG):
                    js = g * NG + k
                    nc.scalar.activation(pss[k], pss[k], AF.Sqrt,
                                         bias=b2_sb[:, js:js + 1])
                for k in range(NG):
                    js = g * NG + k
                    nc.scalar.activation(KT[:, js, :], pss[k], AF.Exp,
                                         scale=EXP_SCALE,
                                         accum_out=cs_sb[:, js:js + 1])

        rs_sb = const.tile([128, NI], F32, tag="rs_sb")
        alpha = const.tile([128, NI], F32, tag="alpha")
        beta = const.tile([128, NI], F32, tag="beta")
        sgen = const.tile([128, NI, D], BF16, tag="sgen")

        # ---- output matmuls, row sums fused on the same stationaries ----
        # Two passes over V, each covering an ic-pair; within a pass the
        # gen half (S_gen + rg) runs first, is evicted, then the pos half.
        NDH = (D + 511) // 512
        vt_pool = ctx.enter_context(tc.tile_pool(name="vt_pool", bufs=3))
        m2pool = ctx.enter_context(tc.tile_pool(name="m2pool", bufs=3,
                                                space="PSUM"))
        rspool = ctx.enter_context(tc.tile_pool(name="rspool", bufs=2,
                                                space="PSUM"))
        for half in range(NI // 2):
            ics = [2 * half, 2 * half + 1]
            mg = {ic: m2pool.tile([128, D], F32, tag="m2h", name=f"mg{ic}")
                  for ic in ics}
            rg = {ic: rspool.tile([128, 1], F32, tag="racc", name=f"rg{ic}")
                  for ic in ics}
            for jb in range(NJG // 4):
                vt = vt_pool.tile([128, 4, D], BF16, tag="vt")
                nc.sync.dma_start(
                    out=vt,
                    in_=targets_bf[jb * 512:(jb + 1) * 512, :].rearrange(
                        "(k p) d -> p k d", p=128))
                for kk in range(4):
                    js = jb * 4 + kk
                    for ic in ics:
                        lhs = KT[:, js, ic * 128:(ic + 1) * 128]
                        for dh in range(NDH):
                            d0 = dh * 512
                            d1 = min(D, d0 + 512)
                            nc.tensor.matmul(mg[ic][:, d0:d1], lhsT=lhs,
                                             rhs=vt[:, kk, d0:d1],
                                             start=(js == 0),
                                             stop=(js == NJG - 1))
                        nc.tensor.matmul(rg[ic], lhsT=lhs, rhs=ones_bf,
                                         start=(js == 0), stop=(js == NJG - 1))
            for ic in ics:
                nc.vector.tensor_scalar_mul(beta[:, ic:ic + 1], rg[ic],
                                            INV_NORM2)
                nc.scalar.copy(sgen[:, ic, :], mg[ic])
            mp = {ic: m2pool.tile([128, D], F32, tag="m2h", name=f"mp{ic}")
                  for ic in ics}
            rp = {ic: rspool.tile([128, 1], F32, tag="racc", name=f"rp{ic}")
                  for ic in ics}
            for jb in range(NJG // 4, NJ // 4):
                vt = vt_pool.tile([128, 4, D], BF16, tag="vt")
                nc.sync.dma_start(
                    out=vt,
                    in_=targets_bf[jb * 512:(jb + 1) * 512, :].rearrange(
                        "(k p) d -> p k d", p=128))
                for kk in range(4):
                    js = jb * 4 + kk
                    for ic in ics:
                        lhs = KT[:, js, ic * 128:(ic + 1) * 128]
                        for dh in range(NDH):
                            d0 = dh * 512
                            d1 = min(D, d0 + 512)
                            nc.tensor.matmul(mp[ic][:, d0:d1], lhsT=lhs,
                                             rhs=vt[:, kk, d0:d1],
                                             start=(js == NJG),
                                             stop=(js == NJ - 1))
                        nc.tensor.matmul(rp[ic], lhsT=lhs, rhs=ones_bf,
                                         start=(js == NJG), stop=(js == NJ - 1))
            for ic in ics:
                nc.vector.tensor_scalar_mul(alpha[:, ic:ic + 1], rp[ic],
                                            INV_NORM2)
            # out = beta*S_pos - alpha*S_gen
            # (alpha here = 1e12*rp scales the GEN half; beta = 1e12*rg
            #  scales the POS half, matching the reference's cross pairing)
            for ic in ics:
                t1 = work.tile([128, D], F32, tag="f32big")
                nc.vector.tensor_scalar_mul(t1, mp[ic], beta[:, ic:ic + 1])
                t2 = work.tile([128, D], F32, tag="f32big")
                nc.vector.tensor_scalar_mul(t2, sgen[:, ic, :],
                                            alpha[:, ic:ic + 1])
                ot = work.tile([128, D], F32, tag="f32big")
                nc.vector.tensor_sub(ot, t1, t2)
                nc.sync.dma_start(out=out[ic * 128:(ic + 1) * 128, :], in_=ot)

        # rs = (alpha + beta) / 1e12, both already in SBUF
        nc.vector.tensor_add(rs_sb, alpha, beta)
        nc.vector.tensor_scalar_mul(rs_sb, rs_sb, 1.0 / INV_NORM2)
        nc.sync.dma_start(out=rs_out[:, :], in_=rs_sb)
        nc.sync.dma_start(out=cs_part[:, :], in_=cs_sb)

    nc.compile()
    return nc


def make_in_maps(gen, pos, G, P, D):
    import ml_dtypes
    T = G + P
    RPC = G // NCORES
    WFILL = T + RPC
    targets = np.concatenate([gen, pos], axis=0).astype(np.float32)
    targets_bf = np.ascontiguousarray(targets.astype(ml_dtypes.bfloat16))
    targets_t_bf = np.ascontiguousarray(targets_bf.T)
    in_maps = []
    p = np.arange(128)
    for c in range(NCORES):
        fill = np.zeros((128, WFILL), ml_dtypes.bfloat16)
        # diagonal entries: q = T + p - c*RPC  (js-independent; the per-js
        # window [T - js*128, +RPC) hits it exactly at the gen-block diagonal)
        q = T + p - c * RPC
        fill[p, q] = BIG
        in_maps.append({
            "gen_rows": np.ascontiguousarray(gen[c * RPC:(c + 1) * RPC]).astype(np.float32),
            "targets_bf": targets_bf,
            "targets_T_bf": targets_t_bf,
            "fill_wide": fill,
        })
    return in_maps


def _exact_numpy_reference(gen, pos):
    """Bit-faithful (float64) fallback for inputs outside the clamped regime."""
    G, D = gen.shape
    gen64 = gen.astype(np.float64)
    pos64 = pos.astype(np.float64)
    tgt = np.concatenate([gen64, pos64], 0)
    d2 = (gen64 * gen64).sum(-1)[:, None] + (tgt * tgt).sum(-1)[None, :] \
        - 2.0 * gen64 @ tgt.T
    dist = np.sqrt(np.maximum(d2, 0.0))
    if D > 10:
        dist = dist / np.sqrt(D)
    idx = np.arange(G)
    dist[idx, idx] = 1e6
    k = np.exp(-dist / TEMP)
    rs = k.sum(-1, keepdims=True)
    cs = k.sum(-2, keepdims=True)
    nk = k / np.sqrt(np.maximum(rs * cs, CLAMP))
    pos_c = nk[:, G:] * nk[:, :G].sum(-1, keepdims=True)
    neg_c = nk[:, :G] * nk[:, G:].sum(-1, keepdims=True)
    return (pos_c @ pos64 - neg_c @ gen64).astype(np.float32)


_NC_CACHE = {}


def _get_nc(G, P, D):
    key = (G, P, D)
    if key not in _NC_CACHE:
        _NC_CACHE[key] = build_nc(G, P, D)
    return _NC_CACHE[key]


def kernel(gen_features, pos_features):
    global LAST_RESULT
    gen = np.asarray(gen_features, dtype=np.float32)
    pos = np.asarray(pos_features, dtype=np.float32)
    G, D = gen.shape
    P = pos.shape[0]
    nc = _get_nc(G, P, D)
    in_maps = make_in_maps(gen, pos, G, P, D)
    res = run_bass_kernel_spmd(nc, in_maps, core_ids=list(range(NCORES)),
                               trace=TRACE)
    LAST_RESULT = res
    out = np.concatenate([res.results[c]["out"] for c in range(NCORES)], axis=0)

    # Clamp-regime guard: the device kernel assumes rs_i*cs_j <= 1e-12
    # everywhere (always true for this problem's data). Verify from the
    # device's own row/column sums; fall back to exact evaluation if not.
    cs_glob = sum(res.results[c]["cs_part"] for c in range(NCORES))
    rs_max = max(float(res.results[c]["rs_out"].max()) for c in range(NCORES))
    if rs_max * float(cs_glob.max()) > 0.25 * CLAMP:
        return _exact_numpy_reference(gen, pos)
    return out.astype(np.float32)


# revision 46
# speedup vs baseline: 1.0829x; 1.0091x over previous
"""Distributed Trainium2 Bass kernel for the DriftingField problem.

Math (reference):
    targets = [gen; pos]                         # [T, D], T = G + P
    d2[i,j] = |gen_i|^2 + |tgt_j|^2 - 2 gen_i.tgt_j
    dist    = sqrt(d2) / sqrt(D); dist[i,i] = 1e6 (gen block diag)
    K       = exp(-dist / TEMP)                  # [G, T]
    nk      = K / sqrt(max(rs_i * cs_j, 1e-12))
    out     = (nk[:,G:] * s_gen) @ pos - (nk[:,:G] * s_pos) @ gen
              s_gen_i = sum_{j<G} nk[i,j], s_pos_i = sum_{j>=G} nk[i,j]

Key numerical fact: for this problem's data (randn features, D=1024,
TEMP=0.05) every K entry is ~exp(-28) ~ 5e-13, so rs_i*cs_j ~ 1e-17 <<
1e-12 and the max() clamp is ALWAYS active: the normalizer is the
constant 1e-6. Then
    out = 1e12 * [ rg_i * (K[:,G:] @ pos) - rp_i * (K[:,:G] @ gen) ]
with rg/rp the per-row sums of K over the gen/pos halves. No cross-core
reduction is needed: the G-sharding is embarrassingly parallel. The
kernel still emits per-core row/column sums; the host checks
max(rs)*max(cs) stays safely under the clamp and falls back to an exact
numpy evaluation if any input ever leaves the clamped regime.

Layout per core (512 gen rows): K^T tiles [j partitions, i free] so that
  - b2_j rides the ACT sqrt bias (per-partition),
  - column-sum partials come free via the Exp activation's accum_out,
  - the output matmuls need no transposition of K.
a2_i is folded into the distance matmul as a K=2 augmentation (hi/lo
bf16 split of a2 against a ones stationary); the gen-block diagonal
DIAG_FILL is folded in as a K=128 matmul of the identity against a
per-core sliding-window mask whose window offset (T - js*128) makes the
diagonal line land js-independently (q = T + p - c*RPC).
Host-side prep (layout/dtype only, no FLOPs moved off-device): concat
targets, transpose, bf16 casts of the matmul operands, the fill mask.
"""

import numpy as np
from contextlib import ExitStack

import concourse.bass as bass
import concourse.bacc as bacc
import concourse.mybir as mybir
import concourse.tile as tile
from concourse.bass_utils import run_bass_kernel_spmd
from concourse.masks import make_identity

F32 = mybir.dt.float32
BF16 = mybir.dt.bfloat16
AF = mybir.ActivationFunctionType

NCORES = 8
TEMP = 0.05
BIG = 1.0e12
CLAMP = 1.0e-12          # reference: max(rs*cs, 1e-12)
INV_NORM2 = 1.0 / CLAMP  # 1e12, the (1/normalizer)^2 when clamped

TRACE = False
LAST_RESULT = None


def build_nc(G, P, D):
    T = G + P
    RPC = G // NCORES          # gen rows per core
    NJ = T // 128              # j-subtiles (target rows)
    NI = RPC // 128            # i-chunks (this core's gen rows)
    ND = D // 128              # d-chunks (feature dim)
    WFILL = T + RPC            # sliding-window fill width
    EXP_SCALE = -1.0 / (TEMP * float(D) ** 0.5)
    NJG = G // 128             # j-subtiles in the gen block

    nc = bacc.Bacc(trn_type="TRN2", num_devices=NCORES)

    gen_rows = nc.dram_tensor("gen_rows", [RPC, D], F32, kind="ExternalInput")
    targets_bf = nc.dram_tensor("targets_bf", [T, D], BF16, kind="ExternalInput")
    targets_T_bf = nc.dram_tensor("targets_T_bf", [D, T], BF16,
                                  kind="ExternalInput")
    fill_wide = nc.dram_tensor("fill_wide", [128, WFILL], BF16,
                               kind="ExternalInput")
    out = nc.dram_tensor("out", [RPC, D], F32, kind="ExternalOutput")
    cs_part = nc.dram_tensor("cs_part", [128, NJ], F32, kind="ExternalOutput")
    rs_out = nc.dram_tensor("rs_out", [128, NI], F32, kind="ExternalOutput")
    a2_dram = nc.dram_tensor("a2_dram", [1, RPC], F32)

    with tile.TileContext(nc) as tc, ExitStack() as ctx:
        const = ctx.enter_context(tc.tile_pool(name="const", bufs=1))
        work = ctx.enter_context(tc.tile_pool(name="work", bufs=3))

        identity = const.tile([128, 128], BF16, tag="identity")
        make_identity(nc, identity)
        ones_bf = const.tile([128, 1], BF16, tag="ones_bf")
        nc.vector.memset(ones_bf, 1.0)

        fill_sb = const.tile([128, WFILL], BF16, tag="fill_sb")
        nc.sync.dma_start(out=fill_sb, in_=fill_wide[:, :])

        b2_sb = const.tile([128, NJ], F32, tag="b2_sb")
        cs_sb = const.tile([128, NJ], F32, tag="cs_sb")
        KT = const.tile([128, NJ, RPC], BF16, tag="KT")
        genT = const.tile([128, ND, RPC], BF16, tag="genT")
        a2_col = const.tile([128, NI], F32, tag="a2_col")
        a2_row = const.tile([1, RPC], F32, tag="a2_row")
        a2_bcast = const.tile([128, RPC], F32, tag="a2_bcast")

        # ---- gen-side prep: a2, cast*(-2), transpose to [d, i] ----
        with tc.tile_pool(name="tpsum", bufs=2, space="PSUM") as tpsum:
            for ic in range(NI):
                gci = work.tile([128, D], F32, tag="f32big")
                nc.sync.dma_start(out=gci, in_=gen_rows[ic * 128:(ic + 1) * 128, :])
                sq_scr = work.tile([128, D], BF16, tag="sqscr")
                nc.scalar.activation(sq_scr, gci, AF.Square,
                                     accum_out=a2_col[:, ic:ic + 1])
                gbf = work.tile([128, D], BF16, tag="bf16big")
                nc.scalar.activation(gbf, gci, AF.Copy, scale=-2.0)
                for dc in range(ND):
                    pt = tpsum.tile([128, 128], BF16, tag="pt")
                    nc.tensor.transpose(pt, gbf[:, dc * 128:(dc + 1) * 128], identity)
                    nc.scalar.copy(genT[:, dc, ic * 128:(ic + 1) * 128], pt)
            # a2 [128, NI] -> row [1, RPC] -> broadcast [128, RPC] via a
            # stride-0 DMA from a DRAM scratch row
            for ic in range(NI):
                nc.sync.dma_start(out=a2_row[0:1, ic * 128:(ic + 1) * 128],
                                  in_=a2_col[:, ic:ic + 1])
            nc.sync.dma_start(out=a2_dram[:, :], in_=a2_row)
            a2d = a2_dram[:, :]
            a2_bc_src = bass.AP(tensor=a2d.tensor, offset=a2d.offset,
                                ap=[[0, 128], a2d.ap[1]])
            nc.sync.dma_start(out=a2_bcast, in_=a2_bc_src)

        # ---- main loop over target row blocks: K^T tiles ----
        # Per group of NG j-subtiles: ACT does only the batched sqrt/exp,
        # DVE does only the b2 square-reduce, PE does everything else
        # (distance matmul + a2 augmentation + diagonal fill matmul).
        NG = 8
        NH = 4  # target-row batch (rows-of-128) for b2 / V loads
        tTb_pool = ctx.enter_context(tc.tile_pool(name="tTb_pool", bufs=2))
        tbf_pool = ctx.enter_context(tc.tile_pool(name="tbf_pool", bufs=2))
        with tc.tile_pool(name="mpsum", bufs=NG, space="PSUM") as mpsum:
            for g in range(NJ // NG):
                j0 = g * NG * 128
                # all 8 d-chunk slices of targets_T for this group: ONE DMA
                tTb = tTb_pool.tile([128, ND, NG * 128], BF16, tag="tTb",
                                    name=f"tTb{g}")
                nc.sync.dma_start(
                    out=tTb,
                    in_=targets_T_bf[:, j0:j0 + NG * 128].rearrange(
                        "(c p) j -> p c j", p=128))
                # b2 via ACT square-reduce, NH rows-of-128 per DMA
                for h in range(NG // NH):
                    jh = j0 + h * NH * 128
                    tbf = tbf_pool.tile([128, NH, D], BF16, tag="tbf",
                                        name=f"tbf{g}_{h}")
                    nc.sync.dma_start(
                        out=tbf,
                        in_=targets_bf[jh:jh + NH * 128, :].rearrange(
                            "(k p) d -> p k d", p=128))
                    for kk in range(NH):
                        js = g * NG + h * NH + kk
                        bscr = work.tile([128, D], BF16, tag="sqscr")
                        nc.scalar.activation(bscr, tbf[:, kk, :], AF.Square,
                                             accum_out=b2_sb[:, js:js + 1])
                pss = []
                for k in range(NG):
                    ps = mpsum.tile([128, RPC], F32, tag="ps", name=f"ps{k}")
                    for dc in range(ND):
                        nc.tensor.matmul(ps,
                                         lhsT=tTb[:, dc, k * 128:(k + 1) * 128],
                                         rhs=genT[:, dc, :],
                                         start=(dc == 0), stop=(dc == ND - 1))
                    pss.append(ps)
                # d2 = (-2 g.t) + a2_i (+BIG on the gen-block diagonal)
                for k in range(NG):
                    js = g * NG + k
                    nc.vector.tensor_add(pss[k], pss[k], a2_bcast)
                    off = T - js * 128
                    nc.vector.tensor_add(pss[k], pss[k], fill_sb[:, off:off + RPC])
                for k in range(NG):
                    js = g * NG + k
                    nc.scalar.activation(pss[k], pss[k], AF.Sqrt,
                                         bias=b2_sb[:, js:js + 1])
                for k in range(NG):
                    js = g * NG + k
                    nc.scalar.activation(KT[:, js, :], pss[k], AF.Exp,
                                         scale=EXP_SCALE)

        # column-sum partials (safety output only): DVE reduces of K^T,
        # scheduled into M2's DVE-idle window
        for js in range(NJ):
            nc.vector.tensor_reduce(cs_sb[:, js:js + 1], KT[:, js, :],
                                    axis=mybir.AxisListType.X,
                                    op=mybir.AluOpType.add)

        rs_sb = const.tile([128, NI], F32, tag="rs_sb")
        alpha = const.tile([128, NI], F32, tag="alpha")
        beta = const.tile([128, NI], F32, tag="beta")
        sgen = const.tile([128, NI, D], BF16, tag="sgen")

        # ---- output matmuls, row sums fused on the same stationaries ----
        # Two passes over V, each covering an ic-pair; within a pass the
        # gen half (S_gen + rg) runs first, is evicted, then the pos half.
        NDH = (D + 511) // 512
        vt_pool = ctx.enter_context(tc.tile_pool(name="vt_pool", bufs=3))
        m2pool = ctx.enter_context(tc.tile_pool(name="m2pool", bufs=3,
                                                space="PSUM"))
        rspool = ctx.enter_context(tc.tile_pool(name="rspool", bufs=2,
                                                space="PSUM"))
        for half in range(NI // 2):
            ics = [2 * half, 2 * half + 1]
            mg = {ic: m2pool.tile([128, D], F32, tag="m2h", name=f"mg{ic}")
                  for ic in ics}
            rg = {ic: rspool.tile([128, 1], F32, tag="racc", name=f"rg{ic}")
                  for ic in ics}
            for jb in range(NJG // 4):
                vt = vt_pool.tile([128, 4, D], BF16, tag="vt")
                nc.sync.dma_start(
                    out=vt,
                    in_=targets_bf[jb * 512:(jb + 1) * 512, :].rearrange(
                        "(k p) d -> p k d", p=128))
                for kk in range(4):
                    js = jb * 4 + kk
                    for ic in ics:
                        lhs = KT[:, js, ic * 128:(ic + 1) * 128]
                        for dh in range(NDH):
                            d0 = dh * 512
                            d1 = min(D, d0 + 512)
                            nc.tensor.matmul(mg[ic][:, d0:d1], lhsT=lhs,
                                             rhs=vt[:, kk, d0:d1],
                                             start=(js == 0),
                                             stop=(js == NJG - 1))
                        nc.tensor.matmul(rg[ic], lhsT=lhs, rhs=ones_bf,
                                         start=(js == 0), stop=(js == NJG - 1))
            for ic in ics:
                nc.vector.tensor_scalar_mul(beta[:, ic:ic + 1], rg[ic],
                                            INV_NORM2)
                nc.scalar.copy(sgen[:, ic, :], mg[ic])
            mp = {ic: m2pool.tile([128, D], F32, tag="m2h", name=f"mp{ic}")
                  for ic in ics}
            rp = {ic: rspool.tile([128, 1], F32, tag="racc", name=f"rp{ic}")
                  for ic in ics}
            for jb in range(NJG // 4, NJ // 4):
                vt = vt_pool.tile([128, 4, D], BF16, tag="vt")
                nc.sync.dma_start(
                    out=vt,
                    in_=targets_bf[jb * 512:(jb + 1) * 512, :].rearrange(
                        "(k p) d -> p k d", p=128))
                for kk in range(4):
                    js = jb * 4 + kk
                    for ic in ics:
                        lhs = KT[:, js, ic * 128:(ic + 1) * 128]
                        for dh in range(NDH):
                            d0 = dh * 512
                            d1 = min(D, d0 + 512)
                            nc.tensor.matmul(mp[ic][:, d0:d1], lhsT=lhs,
                                             rhs=vt[:, kk, d0:d1],
                                             start=(js == NJG),
                                             stop=(js == NJ - 1))
                        nc.tensor.matmul(rp[ic], lhsT=lhs, rhs=ones_bf,
                                         start=(js == NJG), stop=(js == NJ - 1))
            for ic in ics:
                nc.vector.tensor_scalar_mul(alpha[:, ic:ic + 1], rp[ic],
                                            INV_NORM2)
            # out = beta*S_pos - alpha*S_gen
            # (alpha here = 1e12*rp scales the GEN half; beta = 1e12*rg
            #  scales the POS half, matching the reference's cross pairing)
            for ic in ics:
                t1 = work.tile([128, D], F32, tag="f32big")
                nc.vector.tensor_scalar_mul(t1, mp[ic], beta[:, ic:ic + 1])
                t2 = work.tile([128, D], F32, tag="f32big")
                nc.vector.tensor_scalar_mul(t2, sgen[:, ic, :],
                                            alpha[:, ic:ic + 1])
                ot = work.tile([128, D], F32, tag="f32big")
                nc.vector.tensor_sub(ot, t1, t2)
                nc.sync.dma_start(out=out[ic * 128:(ic + 1) * 128, :], in_=ot)

        # rs = (alpha + beta) / 1e12, both already in SBUF
        nc.vector.tensor_add(rs_sb, alpha, beta)
        nc.vector.tensor_scalar_mul(rs_sb, rs_sb, 1.0 / INV_NORM2)
        nc.sync.dma_start(out=rs_out[:, :], in_=rs_sb)
        nc.sync.dma_start(out=cs_part[:, :], in_=cs_sb)

    nc.compile()
    return nc


def make_in_maps(gen, pos, G, P, D):
    import ml_dtypes
    T = G + P
    RPC = G // NCORES
    WFILL = T + RPC
    targets = np.concatenate([gen, pos], axis=0).astype(np.float32)
    targets_bf = np.ascontiguousarray(targets.astype(ml_dtypes.bfloat16))
    targets_t_bf = np.ascontiguousarray(targets_bf.T)
    in_maps = []
    p = np.arange(128)
    for c in range(NCORES):
        fill = np.zeros((128, WFILL), ml_dtypes.bfloat16)
        # diagonal entries: q = T + p - c*RPC  (js-independent; the per-js
        # window [T - js*128, +RPC) hits it exactly at the gen-block diagonal)
        q = T + p - c * RPC
        fill[p, q] = BIG
        in_maps.append({
            "gen_rows": np.ascontiguousarray(gen[c * RPC:(c + 1) * RPC]).astype(np.float32),
            "targets_bf": targets_bf,
            "targets_T_bf": targets_t_bf,
            "fill_wide": fill,
        })
    return in_maps


def _exact_numpy_reference(gen, pos):
    """Bit-faithful (float64) fallback for inputs outside the clamped regime."""
    G, D = gen.shape
    gen64 = gen.astype(np.float64)
    pos64 = pos.astype(np.float64)
    tgt = np.concatenate([gen64, pos64], 0)
    d2 = (gen64 * gen64).sum(-1)[:, None] + (tgt * tgt).sum(-1)[None, :] \
        - 2.0 * gen64 @ tgt.T
    dist = np.sqrt(np.maximum(d2, 0.0))
    if D > 10:
        dist = dist / np.sqrt(D)
    idx = np.arange(G)
    dist[idx, idx] = 1e6
    k = np.exp(-dist / TEMP)
    rs = k.sum(-1, keepdims=True)
    cs = k.sum(-2, keepdims=True)
    nk = k / np.sqrt(np.maximum(rs * cs, CLAMP))
    pos_c = nk[:, G:] * nk[:, :G].sum(-1, keepdims=True)
    neg_c = nk[:, :G] * nk[:, G:].sum(-1, keepdims=True)
    return (pos_c @ pos64 - neg_c @ gen64).astype(np.float32)


_NC_CACHE = {}


def _get_nc(G, P, D):
    key = (G, P, D)
    if key not in _NC_CACHE:
        _NC_CACHE[key] = build_nc(G, P, D)
    return _NC_CACHE[key]


def kernel(gen_features, pos_features):
    global LAST_RESULT
    gen = np.asarray(gen_features, dtype=np.float32)
    pos = np.asarray(pos_features, dtype=np.float32)
    G, D = gen.shape
    P = pos.shape[0]
    nc = _get_nc(G, P, D)
    in_maps = make_in_maps(gen, pos, G, P, D)
    res = run_bass_kernel_spmd(nc, in_maps, core_ids=list(range(NCORES)),
                               trace=TRACE)
    LAST_RESULT = res
    out = np.concatenate([res.results[c]["out"] for c in range(NCORES)], axis=0)

    # Clamp-regime guard: the device kernel assumes rs_i*cs_j <= 1e-12
    # everywhere (always true for this problem's data). Verify from the
    # device's own row/column sums; fall back to exact evaluation if not.
    cs_glob = sum(res.results[c]["cs_part"] for c in range(NCORES))
    rs_max = max(float(res.results[c]["rs_out"].max()) for c in range(NCORES))
    if rs_max * float(cs_glob.max()) > 0.25 * CLAMP:
        return _exact_numpy_reference(gen, pos)
    return out.astype(np.float32)
